# revision 1
# baseline (speedup 1.0000x reference)
"""CostGlobalEncoder TRN2 kernel: conv3x3(324->128) + global HW x HW attention
+ proj + FFN, data-parallel over batch N=8 across 8 NeuronCores.

Self-contained: hardcodes shapes N=8, D=128, H=48, W=64 (HW=3072).
"""
import sys
sys.path.insert(0, '/opt/trn_rl_repo')

import numpy as np
import ml_dtypes

import concourse.bass as bass
import concourse.tile as tile
from concourse import mybir
from concourse.bass_utils import run_bass_kernel_spmd

N, D, H, W = 8, 128, 48, 64
HW = H * W                    # 3072
CIN = 324                     # corr channels
KC = 108                      # conv contraction chunk (324 = 3*108)
NT = 6                        # i-tiles of 512 positions
NP = NT // 2                  # i-tile pairs
TI = 512                      # positions per i-tile
RT = TI // W                  # 8 rows per i-tile
NJ = HW // 128                # 24 j-tiles
SCALE = float(D) ** -0.5

F32 = mybir.dt.float32
BF16 = mybir.dt.bfloat16
AF = mybir.ActivationFunctionType


def _split_multi_waits(nc, max_waits=1):
    """walrus setupSyncWait rejects instructions with several sem-waits;
    hoist extras onto preceding same-engine NOPs (engines run in order)."""
    for fn in nc.m.functions:
        for blk in fn.blocks:
            insts = blk.instructions
            i = 0
            while i < len(insts):
                inst = insts[i]
                si = inst.sync_info
                if si is not None and si.on_wait and len(si.on_wait) > max_waits:
                    waits = list(si.on_wait)
                    extra, keep = waits[:-max_waits], waits[-max_waits:]
                    nops = []
                    while extra:
                        chunk, extra = extra[:max_waits], extra[max_waits:]
                        nop = mybir.InstNoOp(
                            name=f"waitsplit-{nc.next_id()}", ins=[], outs=[])
                        nop.engine = inst.engine
                        nop.sync_info = mybir.SyncInfo(on_wait=chunk, on_update=[])
                        nops.append(nop)
                    inst.sync_info = mybir.SyncInfo(
                        on_wait=keep, on_update=list(si.on_update))
                    blk.instructions = insts = insts[:i] + nops + insts[i:]
                    i += len(nops)
                i += 1


def build_nc(with_bias=True):
    nc = bass.Bass()
    corr = nc.declare_dram_parameter("corr", [CIN, HW], BF16, isOutput=False)
    k_in = nc.declare_dram_parameter("k", [D, HW], BF16, isOutput=False)
    vT = nc.declare_dram_parameter("vT", [128, NJ, D], BF16, isOutput=False)
    wskT = nc.declare_dram_parameter("wskT", [KC, 27, D], BF16, isOutput=False)
    b_sk = nc.declare_dram_parameter("b_sk", [1, D], BF16, isOutput=False)
    wprojT = nc.declare_dram_parameter("wprojT", [2, D, D], BF16, isOutput=False)
    b_proj = nc.declare_dram_parameter("b_proj", [1, D], BF16, isOutput=False)
    wf1T = nc.declare_dram_parameter("wf1T", [D, D], BF16, isOutput=False)
    b_f1 = nc.declare_dram_parameter("b_f1", [D, 1], F32, isOutput=False)
    wf2T = nc.declare_dram_parameter("wf2T", [D, D], BF16, isOutput=False)
    b_f2 = nc.declare_dram_parameter("b_f2", [1, D], BF16, isOutput=False)
    out = nc.declare_dram_parameter("out", [D, HW], F32, isOutput=True)

    with tile.TileContext(nc) as tc:
        with (
            tc.tile_pool(name="const", bufs=1) as cpool,
            tc.tile_pool(name="stage", bufs=2) as spool,
            tc.tile_pool(name="work", bufs=2) as wpool,
            tc.tile_pool(name="qpool", bufs=6) as qpool,
            tc.tile_pool(name="xpool", bufs=7) as xpool,
            tc.tile_pool(name="epool", bufs=27) as epool,
            tc.tile_pool(name="ps_s", bufs=2, space="PSUM") as ps_s,
            tc.tile_pool(name="ps_av", bufs=2, space="PSUM") as ps_av,
            tc.tile_pool(name="ps_conv", bufs=1, space="PSUM") as ps_conv,
        ):
            # ---- load inputs: wskT + corr chunk 0 gate the first matmul ----
            wskT_sb = cpool.tile([KC, 27, D], BF16)
            for c in range(3):
                nc.sync.dma_start(wskT_sb[:, c * 9:(c + 1) * 9, :],
                                  wskT[:, c * 9:(c + 1) * 9, :])
            b_sk_sb = cpool.tile([1, D], BF16)
            nc.sync.dma_start(b_sk_sb[:], b_sk[:])
            corr_pad = []
            for c in range(3):
                cp = cpool.tile([KC, H + 2, W + 2], BF16, name=f"corr_pad{c}")
                nc.vector.memset(cp[:, 0, :], 0.0)
                nc.vector.memset(cp[:, H + 1, :], 0.0)
                nc.vector.memset(cp[:, 1:H + 1, 0:1], 0.0)
                nc.vector.memset(cp[:, 1:H + 1, W + 1:W + 2], 0.0)
                stg = spool.tile([KC, HW], BF16, name="stg")
                nc.sync.dma_start(stg[:], corr[c * KC:(c + 1) * KC, :])
                nc.vector.tensor_copy(
                    cp[:, 1:H + 1, 1:W + 1],
                    stg.rearrange("p (h w) -> p h w", h=H))
                corr_pad.append(cp)
            k_sb = cpool.tile([D, HW], BF16)
            nc.sync.dma_start(k_sb[:], k_in[:])
            # vT_sb[p, t, d] = v[d, t*128+p]
            vT_sb = cpool.tile([128, NJ, D], BF16)
            nc.sync.dma_start(vT_sb[:], vT[:])
            wprojT_sb = cpool.tile([D, 2, D], BF16)
            nc.gpsimd.dma_start(wprojT_sb[:], wprojT.rearrange("c p d -> p c d"))
            wf1T_sb = cpool.tile([D, D], BF16)
            nc.gpsimd.dma_start(wf1T_sb[:], wf1T[:])
            wf2T_sb = cpool.tile([D, D], BF16)
            nc.gpsimd.dma_start(wf2T_sb[:], wf2T[:])
            b_proj_sb = cpool.tile([1, D], BF16)
            nc.gpsimd.dma_start(b_proj_sb[:], b_proj[:])
            b_f1_sb = cpool.tile([D, 1], F32)
            nc.gpsimd.dma_start(b_f1_sb[:], b_f1[:])
            b_f2_sb = cpool.tile([1, D], BF16)
            nc.gpsimd.dma_start(b_f2_sb[:], b_f2[:])
            ones_bf = cpool.tile([128, 1], BF16)
            nc.gpsimd.memset(ones_bf[:], 1.0)
            ones_row = cpool.tile([1, TI], BF16)
            nc.gpsimd.memset(ones_row[:], 1.0)
            ones_bf4 = cpool.tile([4, 128], BF16)
            nc.gpsimd.memset(ones_bf4[:], 1.0)
            warm = cpool.tile([128, 128], BF16)
            nc.gpsimd.memset(warm[:], 0.0)
            # HAM warm-up: keep PE busy during the input DMA wait so the
            # first conv runs at 2.4 GHz (clock-gate releases after ~3.4us)
            ps_w = ps_conv.tile([128, 128], F32, name="ps_w", tag="ps_c")
            for _ in range(90):
                nc.tensor.matmul(ps_w[:], warm[:], warm[:],
                                 start=True, stop=True)

            def conv_mm(p):
                """3x3 conv matmuls for i-tiles (2p, 2p+1); weight-paired."""
                ps_c = ps_conv.tile([D, 2, TI], F32, name="ps_c")
                for c in range(3):
                    for t in range(9):
                        dy, dx = t // 3, t % 3
                        for ii in range(2):
                            y0 = (2 * p + ii) * RT
                            nc.tensor.matmul(
                                ps_c[:, ii, :],
                                wskT_sb[:, c * 9 + t, :],
                                corr_pad[c][:, y0 + dy:y0 + dy + RT,
                                            dx:dx + W],
                                start=(c == 0 and t == 0),
                                stop=(c == 2 and t == 8 and not with_bias))
                if with_bias:
                    for ii in range(2):
                        nc.tensor.matmul(ps_c[:, ii, :], b_sk_sb[:],
                                         ones_row[:], start=False, stop=True)
                return ps_c

            def evac_q(ps_c):
                qs = []
                for ii in range(2):
                    q = qpool.tile([D, TI], BF16, name="q")
                    nc.vector.tensor_copy(q[:], ps_c[:, ii, :])
                    qs.append(q)
                return qs

            def evac_resid(ps_c):
                rs = []
                for ii in range(2):
                    resid = qpool.tile([D, TI], F32, name="resid")
                    nc.vector.tensor_copy(resid[:], ps_c[:, ii, :])
                    rs.append(resid)
                return rs

            def conv_evac(ps_c):
                qs = evac_q(ps_c)
                rs = evac_resid(ps_c)
                return list(zip(rs, qs))

            def s_pair(j, qs):
                t = ps_s.tile([128, 2, TI], F32, name="ps_sj", tag="s")
                for ii in range(2):
                    nc.tensor.matmul(t[:, ii, :],
                                     k_sb[:, j * 128:(j + 1) * 128],
                                     qs[ii][:], start=True, stop=True)
                return t

            def normalize_a(ps_m, ii):
                """evacuate + gather the 4 packed denominator rows."""
                m4 = wpool.tile([128, TI], BF16, name="m4")
                nc.vector.tensor_copy(m4[:], ps_m[:])
                m4g = wpool.tile([4, TI], BF16, name="m4g")
                for g in range(4):
                    nc.sync.dma_start(m4g[g:g + 1, :],
                                      m4[32 * g:32 * g + 1, :])
                return m4g

            def normalize_b(m4g, ps_a):
                """merge+broadcast denominators via gathered [2,TI] matmul."""
                ps_b = ps_s.tile([128, TI], F32, name="ps_b", tag="s")
                nc.tensor.matmul(ps_b[:], ones_bf4[:], m4g[:],
                                 start=True, stop=True)
                rb = wpool.tile([128, TI], F32, name="rb")
                nc.vector.reciprocal(rb[:], ps_b[:])
                attn = wpool.tile([D, TI], BF16, name="attn")
                nc.vector.tensor_mul(attn[:], ps_a[:], rb[:])
                return attn

            def proj(attn, resid, q):
                """1x1 proj on concat([attn, resid]) + bias + resid."""
                ps_p = ps_av.tile([D, TI], F32, name="ps_p", tag="av")
                nc.tensor.matmul(ps_p[:], wprojT_sb[:, 0, :], attn[:],
                                 start=True, stop=False)
                nc.tensor.matmul(ps_p[:], wprojT_sb[:, 1, :], q[:],
                                 start=False, stop=not with_bias)
                if with_bias:
                    nc.tensor.matmul(ps_p[:], b_proj_sb[:], ones_row[:],
                                     start=False, stop=True)
                x = xpool.tile([D, TI], F32, name="x")
                nc.vector.tensor_add(x[:], ps_p[:], resid[:])
                x_bf = xpool.tile([D, TI], BF16, name="x_bf")
                nc.vector.tensor_copy(x_bf[:], x[:])
                return x, x_bf

            def ffn(xv, i):
                x, x_bf = xv
                ps_f1 = ps_s.tile([D, TI], F32, name="ps_f1", tag="s")
                nc.tensor.matmul(ps_f1[:], wf1T_sb[:], x_bf[:],
                                 start=True, stop=True)
                h1 = wpool.tile([D, TI], BF16, name="h1")
                nc.scalar.activation(h1[:], ps_f1[:], AF.Gelu, bias=b_f1_sb[:])
                ps_f2 = ps_s.tile([D, TI], F32, name="ps_f2", tag="s")
                nc.tensor.matmul(ps_f2[:], wf2T_sb[:], h1[:],
                                 start=True, stop=not with_bias)
                if with_bias:
                    nc.tensor.matmul(ps_f2[:], b_f2_sb[:], ones_row[:],
                                     start=False, stop=True)
                o = wpool.tile([D, TI], F32, name="o")
                nc.vector.tensor_add(o[:], ps_f2[:], x[:])
                nc.sync.dma_start(out[:, i * TI:(i + 1) * TI], o[:])

            xs = [None] * NT
            rq_pair = conv_evac(conv_mm(0))
            qpair = [rq_pair[0][1], rq_pair[1][1]]
            prime = s_pair(0, qpair)
            for p in range(NP):
                i0, i1 = 2 * p, 2 * p + 1
                # ---- attention j-loop, software-pipelined by one j ----
                ps_a0 = ps_av.tile([D, TI], F32, name="ps_a0", tag="av")
                ps_a1 = ps_av.tile([D, TI], F32, name="ps_a1", tag="av")
                ps_sj = prime
                e_tiles = []
                for j in range(NJ):
                    ps_nxt = s_pair(j + 1, qpair) if j + 1 < NJ else None
                    e = epool.tile([128, 2, TI], BF16, name="e")
                    nc.scalar.activation(e[:], ps_sj[:], AF.Exp, scale=SCALE)
                    for ii, ps_aa in ((0, ps_a0), (1, ps_a1)):
                        nc.tensor.matmul(ps_aa[:], vT_sb[:, j, :],
                                         e[:, ii, :],
                                         start=(j == 0), stop=(j == NJ - 1))
                    e_tiles.append(e)
                    ps_sj = ps_nxt

                # ---- softmax denominators: col-packed ones-matmuls ----
                m4gs = []
                for ii in range(2):
                    ps_m = ps_s.tile([128, TI], F32, name="ps_m", tag="s")
                    for g4 in range(NJ // 4):
                        for g in range(4):
                            nc.tensor.matmul(
                                ps_m[32 * g:32 * g + 1, :], ones_bf[:, 0:1],
                                e_tiles[g4 * 4 + g][:, ii, :],
                                start=(g4 == 0), stop=(g4 == NJ // 4 - 1),
                                tile_position=(0, 32 * g))
                    m4gs.append(normalize_a(ps_m, ii))

                rq_prev = rq_pair
                ps_c_next = conv_mm(p + 1) if p + 1 < NP else None
                qs_next = evac_q(ps_c_next) if ps_c_next is not None else None

                last = p == NP - 1
                attn0 = normalize_b(m4gs[0], ps_a0)
                if last:
                    ffn(xs[0], 0)
                    ffn(xs[1], 1)
                xs[2 * p] = proj(attn0, rq_prev[0][0], rq_prev[0][1])
                if ps_c_next is not None:
                    # prime the next pair's first S now: the freed s-slot is
                    # available and this unblocks the next j-loop early
                    qpair = qs_next
                    prime = s_pair(0, qpair)
                    rq_pair = list(zip(evac_resid(ps_c_next), qs_next))
                attn1 = normalize_b(m4gs[1], ps_a1)
                if last:
                    ffn(xs[2], 2)
                    ffn(xs[3], 3)
                xs[2 * p + 1] = proj(attn1, rq_prev[1][0], rq_prev[1][1])

            # ---- FFN for the last pair ----
            for i in range(2 * (NP - 1), NT):
                ffn(xs[i], i)

    _split_multi_waits(nc)
    return nc


_NC = {}


def _get_nc(with_bias=True):
    if with_bias not in _NC:
        _NC[with_bias] = build_nc(with_bias)
    return _NC[with_bias]


def _prep_core(corr, k, v, w_sk, b_sk, w_proj, b_proj, w_ffn1, b_ffn1,
               w_ffn2, b_ffn2):
    bf = ml_dtypes.bfloat16
    wskT = np.empty((KC, 27, D), dtype=bf)
    for c in range(3):
        for t in range(9):
            dy, dx = t // 3, t % 3
            wskT[:, c * 9 + t, :] = \
                w_sk[:, c * KC:(c + 1) * KC, dy, dx].T.astype(bf)
    vT = v.reshape(D, HW).T.reshape(NJ, 128, D).transpose(1, 0, 2)
    return {
        "corr": corr.reshape(CIN, HW).astype(bf),
        "k": k.reshape(D, HW).astype(bf),
        "vT": np.ascontiguousarray(vT).astype(bf),
        "wskT": wskT,
        "b_sk": b_sk.reshape(1, D).astype(bf),
        "wprojT": np.ascontiguousarray(
            w_proj.reshape(D, 2 * D).T.reshape(2, D, D)).astype(bf),
        "b_proj": b_proj.reshape(1, D).astype(bf),
        "wf1T": np.ascontiguousarray(w_ffn1.reshape(D, D).T).astype(bf),
        "b_f1": b_ffn1.reshape(D, 1).astype(np.float32),
        "wf2T": np.ascontiguousarray(w_ffn2.reshape(D, D).T).astype(bf),
        "b_f2": b_ffn2.reshape(1, D).astype(bf),
    }


def make_in_maps(corr, k, v, w_sk, b_sk, w_proj, b_proj, w_ffn1, b_ffn1,
                 w_ffn2, b_ffn2):
    corr = np.asarray(corr, dtype=np.float32)
    k = np.asarray(k, dtype=np.float32)
    v = np.asarray(v, dtype=np.float32)
    return [
        _prep_core(corr[i], k[i], v[i], np.asarray(w_sk, np.float32),
                   np.asarray(b_sk, np.float32),
                   np.asarray(w_proj, np.float32),
                   np.asarray(b_proj, np.float32),
                   np.asarray(w_ffn1, np.float32),
                   np.asarray(b_ffn1, np.float32),
                   np.asarray(w_ffn2, np.float32),
                   np.asarray(b_ffn2, np.float32))
        for i in range(N)
    ]


def kernel(corr, k, v, w_sk, b_sk, w_proj, b_proj, w_ffn1, b_ffn1,
           w_ffn2, b_ffn2):
    with_bias = bool(np.any(np.asarray(b_proj)) or np.any(np.asarray(b_ffn2))
                     or np.any(np.asarray(b_sk)))
    nc = _get_nc(with_bias)
    in_maps = make_in_maps(corr, k, v, w_sk, b_sk, w_proj, b_proj,
                           w_ffn1, b_ffn1, w_ffn2, b_ffn2)
    res = run_bass_kernel_spmd(nc, in_maps, list(range(N)))
    out = np.stack([res.results[i]["out"].reshape(D, H, W) for i in range(N)])
    return out.astype(np.float32)



# revision 14
# speedup vs baseline: 1.1227x; 1.1227x over previous
"""CostGlobalEncoder TRN2 kernel: conv3x3(324->128) + global HW x HW attention
+ proj + FFN, data-parallel over batch N=8 across 8 NeuronCores.

Self-contained: hardcodes shapes N=8, D=128, H=48, W=64 (HW=3072).

Structure (per core, one batch sample):
  - conv feeds q; S = k^T q per 128-key j-tile; exp on ScalarE (Act) writes
    fp8 e-tiles; AV accumulates with fp8 DoubleRow matmuls over j-pairs;
    softmax denominators via ones-matmuls col-packed 4-wide into one PSUM
    bank (rows 32g + 16*ii); denominator merge+broadcast via one mask
    matmul; reciprocal_approx_fast; proj; FFN (all gelus at the tail so
    the Act table set switches exactly once).
  - The j-loop is paced by the Act engine's exp; conv for the next pair,
    denominator matmuls, and the previous pair's normalize/proj are
    interleaved into the loop so the PE never idles long enough to lose
    the HAM 2.4 GHz clock.
"""
import sys
sys.path.insert(0, '/opt/trn_rl_repo')

import numpy as np
import ml_dtypes

import concourse.bass as bass
import concourse.tile as tile
from concourse import mybir
from concourse.bass_utils import run_bass_kernel_spmd

N, D, H, W = 8, 128, 48, 64
HW = H * W                    # 3072
CIN = 324                     # corr channels
KC = 108                      # conv contraction chunk (324 = 3*108)
NT = 6                        # i-tiles of 512 positions
NP = NT // 2                  # i-tile pairs
TI = 512                      # positions per i-tile
RT = TI // W                  # 8 rows per i-tile
NJ = HW // 128                # 24 j-tiles
NJP = NJ // 2                 # 12 j-tile pairs (fp8 DoubleRow)
SCALE = float(D) ** -0.5
EBIAS = -3.0                  # exp bias keeps fp8 e-values < 240 (TRN e4m3 inf)

F32 = mybir.dt.float32
BF16 = mybir.dt.bfloat16
F8 = mybir.dt.float8e4
AF = mybir.ActivationFunctionType
DR = mybir.MatmulPerfMode.DoubleRow


def _split_multi_waits(nc, max_waits=1):
    """walrus setupSyncWait rejects instructions with several sem-waits;
    hoist extras onto preceding same-engine NOPs (engines run in order)."""
    for fn in nc.m.functions:
        for blk in fn.blocks:
            insts = blk.instructions
            i = 0
            while i < len(insts):
                inst = insts[i]
                si = inst.sync_info
                if si is not None and si.on_wait and len(si.on_wait) > max_waits:
                    waits = list(si.on_wait)
                    extra, keep = waits[:-max_waits], waits[-max_waits:]
                    nops = []
                    while extra:
                        chunk, extra = extra[:max_waits], extra[max_waits:]
                        nop = mybir.InstNoOp(
                            name=f"waitsplit-{nc.next_id()}", ins=[], outs=[])
                        nop.engine = inst.engine
                        nop.sync_info = mybir.SyncInfo(on_wait=chunk, on_update=[])
                        nops.append(nop)
                    inst.sync_info = mybir.SyncInfo(
                        on_wait=keep, on_update=list(si.on_update))
                    blk.instructions = insts = insts[:i] + nops + insts[i:]
                    i += len(nops)
                i += 1


def build_nc(with_bias=True):
    nc = bass.Bass()
    corr = nc.declare_dram_parameter("corr", [CIN, HW], BF16, isOutput=False)
    k_in = nc.declare_dram_parameter("k", [D, HW], BF16, isOutput=False)
    vT2 = nc.declare_dram_parameter("vT2", [128, NJP, 2, D], F8, isOutput=False)
    wskT = nc.declare_dram_parameter("wskT", [KC, 27, D], BF16, isOutput=False)
    b_sk = nc.declare_dram_parameter("b_sk", [1, D], BF16, isOutput=False)
    wprojT = nc.declare_dram_parameter("wprojT", [2, D, D], BF16, isOutput=False)
    b_proj = nc.declare_dram_parameter("b_proj", [1, D], BF16, isOutput=False)
    wf1T = nc.declare_dram_parameter("wf1T", [D, D], BF16, isOutput=False)
    b_f1 = nc.declare_dram_parameter("b_f1", [D, 1], F32, isOutput=False)
    wf2T = nc.declare_dram_parameter("wf2T", [D, D], BF16, isOutput=False)
    b_f2 = nc.declare_dram_parameter("b_f2", [1, D], BF16, isOutput=False)
    out = nc.declare_dram_parameter("out", [D, HW], F32, isOutput=True)

    with tile.TileContext(nc) as tc:
        with (
            tc.tile_pool(name="const", bufs=1) as cpool,
            tc.tile_pool(name="stage", bufs=2) as spool,
            tc.tile_pool(name="work", bufs=3) as wpool,
            tc.tile_pool(name="qpool", bufs=4) as qpool,
            tc.tile_pool(name="xpool", bufs=7) as xpool,
            tc.tile_pool(name="epool", bufs=6) as epool,
            tc.tile_pool(name="ps_s", bufs=2, space="PSUM") as ps_s,
            tc.tile_pool(name="ps_av", bufs=2, space="PSUM") as ps_av,
            tc.tile_pool(name="ps_c", bufs=1, space="PSUM") as ps_cp,
            tc.tile_pool(name="ps_m", bufs=1, space="PSUM") as ps_mp,
        ):
            # ---- input DMAs: wskT + corr chunks gate the first conv ----
            wskT_sb = cpool.tile([KC, 27, D], BF16)
            for c in range(3):
                nc.sync.dma_start(wskT_sb[:, c * 9:(c + 1) * 9, :],
                                  wskT[:, c * 9:(c + 1) * 9, :])
            corr_pad = []
            for c in range(3):
                cp = cpool.tile([KC, H + 2, W + 2], BF16, name=f"corr_pad{c}")
                nc.vector.memset(cp[:, 0, :], 0.0)
                nc.vector.memset(cp[:, H + 1, :], 0.0)
                nc.vector.memset(cp[:, 1:H + 1, 0:1], 0.0)
                nc.vector.memset(cp[:, 1:H + 1, W + 1:W + 2], 0.0)
                stg = spool.tile([KC, HW], BF16, name="stg")
                nc.sync.dma_start(stg[:], corr[c * KC:(c + 1) * KC, :])
                nc.vector.tensor_copy(
                    cp[:, 1:H + 1, 1:W + 1],
                    stg.rearrange("p (h w) -> p h w", h=H))
                corr_pad.append(cp)
            k_sb = cpool.tile([D, HW], BF16)
            nc.sync.dma_start(k_sb[:], k_in[:])
            # vT2_sb[p, jp, jj, d] = v[d, (2*jp+jj)*128 + p] in fp8
            vT2_sb = cpool.tile([128, NJP, 2, D], F8)
            nc.sync.dma_start(vT2_sb[:], vT2[:])
            wprojT_sb = cpool.tile([D, 2, D], BF16)
            nc.gpsimd.dma_start(wprojT_sb[:], wprojT.rearrange("c p d -> p c d"))
            wf1T_sb = cpool.tile([D, D], BF16)
            nc.gpsimd.dma_start(wf1T_sb[:], wf1T[:])
            wf2T_sb = cpool.tile([D, D], BF16)
            nc.gpsimd.dma_start(wf2T_sb[:], wf2T[:])
            b_sk_sb = cpool.tile([1, D], BF16)
            nc.gpsimd.dma_start(b_sk_sb[:], b_sk[:])
            b_proj_sb = cpool.tile([1, D], BF16)
            nc.gpsimd.dma_start(b_proj_sb[:], b_proj[:])
            b_f1_sb = cpool.tile([D, 1], F32)
            nc.gpsimd.dma_start(b_f1_sb[:], b_f1[:])
            b_f2_sb = cpool.tile([1, D], BF16)
            nc.gpsimd.dma_start(b_f2_sb[:], b_f2[:])
            ones_f8 = cpool.tile([128, 1], F8)
            nc.gpsimd.memset(ones_f8[:], 1.0)
            ones_row = cpool.tile([1, TI], BF16)
            nc.gpsimd.memset(ones_row[:], 1.0)
            # mask[r, ii, :] = 1 iff row r holds an ii-denominator partial
            # (ii0 partials live in rows {0,32}, ii1 in rows {64,96})
            mask = cpool.tile([128, 2, 128], BF16)
            nc.gpsimd.memset(mask[:], 0.0)
            for t in range(2):
                for ii in range(2):
                    r = 64 * ii + 32 * t
                    nc.vector.memset(mask[r:r + 1, ii, :], 1.0)
            warm = cpool.tile([128, 128], BF16)
            nc.gpsimd.memset(warm[:], 0.0)
            ebias_sb = cpool.tile([128, 1], F32)
            nc.gpsimd.memset(ebias_sb[:], EBIAS)
            # preload the exp table set during the DMA wait
            tdummy = cpool.tile([1, 16], BF16)
            nc.scalar.activation(tdummy[:], ones_row[0:1, 0:16], AF.Exp)
            # HAM warm-up: keep PE busy while corr DMA lands so conv runs
            # at 2.4 GHz as early as possible
            ps_w = ps_cp.tile([128, 128], F32, name="ps_w", tag="c")
            for _ in range(24):
                nc.tensor.matmul(ps_w[:], warm[:], warm[:],
                                 start=True, stop=True)

            def conv_gen(p):
                """3x3 conv for i-tiles (2p, 2p+1), one [D,TI] psum bank,
                ii-serial. Yields None after each PE matmul / evac step,
                then yields the [(resid, q), (resid, q)] result forever."""
                outs = []
                for ii in range(2):
                    ps_c = ps_cp.tile([D, TI], F32, name="ps_c", tag="c")
                    y0 = (2 * p + ii) * RT
                    for c in range(3):
                        for t in range(9):
                            dy, dx = t // 3, t % 3
                            nc.tensor.matmul(
                                ps_c[:],
                                wskT_sb[:, c * 9 + t, :],
                                corr_pad[c][:, y0 + dy:y0 + dy + RT,
                                            dx:dx + W],
                                start=(c == 0 and t == 0),
                                stop=(c == 2 and t == 8 and not with_bias))
                            yield None
                    if with_bias:
                        nc.tensor.matmul(ps_c[:], b_sk_sb[:], ones_row[:],
                                         start=False, stop=True)
                        yield None
                    q = qpool.tile([D, TI], BF16, name="q", tag="q")
                    nc.vector.tensor_copy(q[:], ps_c[:])
                    resid = qpool.tile([D, TI], F32, name="resid", tag="r")
                    nc.vector.tensor_copy(resid[:], ps_c[:])
                    outs.append((resid, q))
                    yield None
                while True:
                    yield outs

            def run_conv(gen):
                while True:
                    r = next(gen)
                    if r is not None:
                        return r

            def s_pair(j, qs):
                t = ps_s.tile([128, 2, TI], F32, name="ps_sj", tag="s")
                for ii in range(2):
                    nc.tensor.matmul(t[:, ii, :],
                                     k_sb[:, j * 128:(j + 1) * 128],
                                     qs[ii][:], start=True, stop=True)
                return t

            def ones_group(ps_m, g4, e2s):
                """denominator partial sums for e-tiles j=4*g4..4*g4+3:
                fp8 ones-matmuls col-packed 4-wide per j-pair tile;
                ii0 -> rows {0,32}, ii1 -> {64,96}. ps_m was
                zero-initialized by a start=True matmul, so always
                accumulate."""
                for t in range(2):
                    for ii in range(2):
                        for jj in range(2):
                            r = 64 * ii + 32 * jj
                            nc.tensor.matmul(
                                ps_m[r:r + 1, :], ones_f8[:, 0:1],
                                e2s[t][:, jj, ii, :],
                                start=False,
                                stop=(g4 == NJ // 4 - 1 and t == 1),
                                skip_group_check=True,
                                tile_position=(0, r))

            def new_ps_m():
                """fresh denominator bank, zeroed by a 0-weights matmul so
                the mask-merge never touches stale PSUM garbage."""
                ps_m = ps_mp.tile([128, TI], F32, name="ps_m", tag="m")
                nc.tensor.matmul(ps_m[:], warm[:], k_sb[:, 0:TI],
                                 start=True, stop=True)
                return ps_m

            def merge_den(m4, ii):
                """ps_b[o, q] = sum_r mask[r, ii, o] * m4[r, q]
                = broadcast denominator; then fast reciprocal."""
                ps_b = ps_mp.tile([128, TI], F32, name="ps_b", tag="m")
                nc.tensor.matmul(ps_b[:], mask[:, ii, :], m4[:],
                                 start=True, stop=True)
                rb = wpool.tile([128, TI], F32, name="rb", tag="rb")
                nc.vector.reciprocal(rb[:], ps_b[:])
                return rb

            def proj(attn, resid, q):
                """1x1 proj on concat([attn, resid]) + bias + resid."""
                ps_p = ps_mp.tile([D, TI], F32, name="ps_p", tag="m")
                nc.tensor.matmul(ps_p[:], wprojT_sb[:, 0, :], attn[:],
                                 start=True, stop=False)
                nc.tensor.matmul(ps_p[:], wprojT_sb[:, 1, :], q[:],
                                 start=False, stop=not with_bias)
                if with_bias:
                    nc.tensor.matmul(ps_p[:], b_proj_sb[:], ones_row[:],
                                     start=False, stop=True)
                x = xpool.tile([D, TI], F32, name="x", tag="x")
                nc.vector.tensor_add(x[:], ps_p[:], resid[:])
                x_bf = xpool.tile([D, TI], BF16, name="x_bf", tag="xb")
                nc.vector.tensor_copy(x_bf[:], x[:])
                return x, x_bf

            def normalize_proj(m4, av, rq, ii):
                rb = merge_den(m4, ii)
                attn = wpool.tile([D, TI], BF16, name="attn", tag="at")
                nc.vector.tensor_mul(attn[:], av[ii][:], rb[:])
                return proj(attn, rq[ii][0], rq[ii][1])

            def ffn(xv, i):
                x, x_bf = xv
                ps_f1 = ps_s.tile([D, TI], F32, name="ps_f1", tag="s")
                nc.tensor.matmul(ps_f1[:], wf1T_sb[:], x_bf[:],
                                 start=True, stop=True)
                h1 = wpool.tile([D, TI], BF16, name="h1", tag="h1")
                nc.scalar.activation(h1[:], ps_f1[:], AF.Gelu, bias=b_f1_sb[:])
                ps_f2 = ps_s.tile([D, TI], F32, name="ps_f2", tag="s")
                nc.tensor.matmul(ps_f2[:], wf2T_sb[:], h1[:],
                                 start=True, stop=not with_bias)
                if with_bias:
                    nc.tensor.matmul(ps_f2[:], b_f2_sb[:], ones_row[:],
                                     start=False, stop=True)
                o = wpool.tile([D, TI], F32, name="o", tag="o")
                nc.vector.tensor_add(o[:], ps_f2[:], x[:])
                nc.sync.dma_start(out[:, i * TI:(i + 1) * TI], o[:])

            # ---- prologue: conv pair 0 + S(0), S(1) ----
            rq = run_conv(conv_gen(0))
            qpair = [rq[0][1], rq[1][1]]
            ps_s0 = s_pair(0, qpair)
            ps_s1 = s_pair(1, qpair)

            xs = [None] * NT
            prev = None              # (m4, av, rq) of pair p-1
            for p in range(NP):
                last = p == NP - 1
                cgen = conv_gen(p + 1) if not last else None
                cdone = None
                qnext = None
                # boundary leftovers from pair p-1, injected into early j's
                inject = []
                if prev is not None:
                    pm4, pav, prq = prev

                    def mk_norm(ii, pp=p):
                        def go():
                            xs[2 * (pp - 1) + ii] = normalize_proj(
                                pm4, pav, prq, ii)
                        return go
                    inject = [mk_norm(0), mk_norm(1)]

                ps_m = None
                ps_sj, ps_snx = ps_s0, ps_s1
                e2 = None
                e2_hist = []
                ps_a = [ps_av.tile([D, TI], F32, name=f"ps_a{ii}", tag="av")
                        for ii in range(2)]
                for j in range(NJ):
                    jp, jj = j // 2, j % 2
                    if jj == 0:
                        e2 = epool.tile([128, 2, 2, TI], F8, name="e2",
                                        tag="e")
                        e2_hist.append(e2)
                    # Act: the pacing instruction
                    nc.scalar.activation(e2[:, jj, :, :], ps_sj[:], AF.Exp,
                                         scale=SCALE, bias=ebias_sb[:])
                    # PE work for this j, in rough priority order
                    if j < NJ - 2:
                        ps_sj, ps_snx = ps_snx, s_pair(j + 2, qpair)
                    elif j == NJ - 2:
                        ps_sj = ps_snx
                        if not last:
                            cdone = run_conv(cgen)
                            cgen = None
                            qnext = [cdone[0][1], cdone[1][1]]
                            ps_s0 = s_pair(0, qnext)
                    else:
                        if not last:
                            ps_s1 = s_pair(1, qnext)
                    if jj == 1:
                        for ii in range(2):
                            nc.tensor.matmul(
                                ps_a[ii][:], vT2_sb[:, jp, :, :],
                                e2[:, :, ii, :],
                                start=(jp == 0), stop=(jp == NJP - 1),
                                perf_mode=DR)
                    if j % 4 == 0 and j > 0:
                        if ps_m is None:
                            ps_m = new_ps_m()
                        ones_group(ps_m, j // 4 - 1, e2_hist[-3:-1])
                    if inject:
                        inject.pop(0)()
                    if cgen is not None:
                        for _ in range(3):
                            next(cgen)
                # ---- boundary: finish denominators, evacuate AV ----
                ones_group(ps_m, NJ // 4 - 1, e2_hist[-2:])
                m4 = wpool.tile([128, TI], BF16, name="m4", tag="m4")
                nc.vector.tensor_copy(m4[:], ps_m[:])
                av = []
                for ii in range(2):
                    a = qpool.tile([D, TI], F32, name="av_sb", tag="avs")
                    nc.vector.tensor_copy(a[:], ps_a[ii][:])
                    av.append(a)
                prev = (m4, av, rq)
                if last:
                    for ii in range(2):
                        xs[2 * p + ii] = normalize_proj(m4, av, rq, ii)
                else:
                    rq = cdone
                    qpair = qnext

            # ---- FFN tail: all gelus after all exps (one table switch) ----
            for i in range(NT):
                ffn(xs[i], i)

    _split_multi_waits(nc)
    return nc


_NC = {}


def _get_nc(with_bias=True):
    if with_bias not in _NC:
        _NC[with_bias] = build_nc(with_bias)
    return _NC[with_bias]


def _prep_core(corr, k, v, w_sk, b_sk, w_proj, b_proj, w_ffn1, b_ffn1,
               w_ffn2, b_ffn2):
    bf = ml_dtypes.bfloat16
    f8 = ml_dtypes.float8_e4m3
    wskT = np.empty((KC, 27, D), dtype=bf)
    for c in range(3):
        for t in range(9):
            dy, dx = t // 3, t % 3
            wskT[:, c * 9 + t, :] = \
                w_sk[:, c * KC:(c + 1) * KC, dy, dx].T.astype(bf)
    vT = v.reshape(D, HW).T.reshape(NJ, 128, D).transpose(1, 0, 2)
    vT2 = np.ascontiguousarray(vT).astype(f8).reshape(128, NJP, 2, D)
    return {
        "corr": corr.reshape(CIN, HW).astype(bf),
        "k": k.reshape(D, HW).astype(bf),
        "vT2": vT2,
        "wskT": wskT,
        "b_sk": b_sk.reshape(1, D).astype(bf),
        "wprojT": np.ascontiguousarray(
            w_proj.reshape(D, 2 * D).T.reshape(2, D, D)).astype(bf),
        "b_proj": b_proj.reshape(1, D).astype(bf),
        "wf1T": np.ascontiguousarray(w_ffn1.reshape(D, D).T).astype(bf),
        "b_f1": b_ffn1.reshape(D, 1).astype(np.float32),
        "wf2T": np.ascontiguousarray(w_ffn2.reshape(D, D).T).astype(bf),
        "b_f2": b_ffn2.reshape(1, D).astype(bf),
    }


def make_in_maps(corr, k, v, w_sk, b_sk, w_proj, b_proj, w_ffn1, b_ffn1,
                 w_ffn2, b_ffn2):
    corr = np.asarray(corr, dtype=np.float32)
    k = np.asarray(k, dtype=np.float32)
    v = np.asarray(v, dtype=np.float32)
    return [
        _prep_core(corr[i], k[i], v[i], np.asarray(w_sk, np.float32),
                   np.asarray(b_sk, np.float32),
                   np.asarray(w_proj, np.float32),
                   np.asarray(b_proj, np.float32),
                   np.asarray(w_ffn1, np.float32),
                   np.asarray(b_ffn1, np.float32),
                   np.asarray(w_ffn2, np.float32),
                   np.asarray(b_ffn2, np.float32))
        for i in range(N)
    ]


def kernel(corr, k, v, w_sk, b_sk, w_proj, b_proj, w_ffn1, b_ffn1,
           w_ffn2, b_ffn2):
    with_bias = bool(np.any(np.asarray(b_proj)) or np.any(np.asarray(b_ffn2))
                     or np.any(np.asarray(b_sk)))
    nc = _get_nc(with_bias)
    in_maps = make_in_maps(corr, k, v, w_sk, b_sk, w_proj, b_proj,
                           w_ffn1, b_ffn1, w_ffn2, b_ffn2)
    res = run_bass_kernel_spmd(nc, in_maps, list(range(N)))
    out = np.stack([res.results[i]["out"].reshape(D, H, W) for i in range(N)])
    return out.astype(np.float32)


# revision 17
# speedup vs baseline: 1.1693x; 1.0415x over previous
"""CostGlobalEncoder TRN2 kernel: conv3x3(324->128) + global HW x HW attention
+ proj + FFN, data-parallel over batch N=8 across 8 NeuronCores.

Self-contained: hardcodes shapes N=8, D=128, H=48, W=64 (HW=3072).

Structure (per core, one batch sample):
  - conv feeds q; S = k^T q per 128-key j-tile; exp on ScalarE writes fp8
    e-tiles; AV accumulates with fp8 DoubleRow matmuls over j-pairs;
    softmax denominators via fp8 ones-matmuls col-packed 4-wide into one
    PSUM bank; denominator merge+broadcast via one mask matmul.
  - Normalization is deferred past the proj matmul (a per-column scale
    commutes through it): x = (Wp0 @ av_unnorm) * (1/den) + resid2 where
    resid2 = resid + Wp1 @ q is precomputed at conv time. The PE therefore
    never waits on the reciprocal.
  - conv for the next pair and the previous pair's normalize/proj are
    interleaved into the Act-paced j-loop; all FFN gelus run at the tail
    so the Act table set switches exactly once.
"""
import sys
sys.path.insert(0, '/opt/trn_rl_repo')

import numpy as np
import ml_dtypes

import concourse.bass as bass
import concourse.tile as tile
from concourse import mybir
from concourse.bass_utils import run_bass_kernel_spmd

N, D, H, W = 8, 128, 48, 64
HW = H * W                    # 3072
CIN = 324                     # corr channels
KC = 108                      # conv contraction chunk (324 = 3*108)
NT = 6                        # i-tiles of 512 positions
NP = NT // 2                  # i-tile pairs
TI = 512                      # positions per i-tile
RT = TI // W                  # 8 rows per i-tile
NJ = HW // 128                # 24 j-tiles
NJP = NJ // 2                 # 12 j-tile pairs (fp8 DoubleRow)
SCALE = float(D) ** -0.5
EBIAS = -3.0                  # exp bias keeps fp8 e-values < 240 (TRN e4m3 inf)

F32 = mybir.dt.float32
BF16 = mybir.dt.bfloat16
F8 = mybir.dt.float8e4
AF = mybir.ActivationFunctionType
DR = mybir.MatmulPerfMode.DoubleRow


def _split_multi_waits(nc, max_waits=1):
    """walrus setupSyncWait rejects instructions with several sem-waits;
    hoist extras onto preceding same-engine NOPs (engines run in order)."""
    for fn in nc.m.functions:
        for blk in fn.blocks:
            insts = blk.instructions
            i = 0
            while i < len(insts):
                inst = insts[i]
                si = inst.sync_info
                if si is not None and si.on_wait and len(si.on_wait) > max_waits:
                    waits = list(si.on_wait)
                    extra, keep = waits[:-max_waits], waits[-max_waits:]
                    nops = []
                    while extra:
                        chunk, extra = extra[:max_waits], extra[max_waits:]
                        nop = mybir.InstNoOp(
                            name=f"waitsplit-{nc.next_id()}", ins=[], outs=[])
                        nop.engine = inst.engine
                        nop.sync_info = mybir.SyncInfo(on_wait=chunk, on_update=[])
                        nops.append(nop)
                    inst.sync_info = mybir.SyncInfo(
                        on_wait=keep, on_update=list(si.on_update))
                    blk.instructions = insts = insts[:i] + nops + insts[i:]
                    i += len(nops)
                i += 1


def build_nc(with_bias=True):
    nc = bass.Bass()
    corr = nc.declare_dram_parameter("corr", [CIN, HW], BF16, isOutput=False)
    k_in = nc.declare_dram_parameter("k", [D, HW], BF16, isOutput=False)
    vT2 = nc.declare_dram_parameter("vT2", [128, NJP, 2, D], F8, isOutput=False)
    wskT = nc.declare_dram_parameter("wskT", [KC, 27, D], BF16, isOutput=False)
    b_sk = nc.declare_dram_parameter("b_sk", [1, D], BF16, isOutput=False)
    wprojT = nc.declare_dram_parameter("wprojT", [2, D, D], BF16, isOutput=False)
    b_proj = nc.declare_dram_parameter("b_proj", [1, D], BF16, isOutput=False)
    wf1T = nc.declare_dram_parameter("wf1T", [D, D], BF16, isOutput=False)
    b_f1 = nc.declare_dram_parameter("b_f1", [D, 1], F32, isOutput=False)
    wf2T = nc.declare_dram_parameter("wf2T", [D, D], BF16, isOutput=False)
    b_f2 = nc.declare_dram_parameter("b_f2", [1, D], BF16, isOutput=False)
    ident = nc.declare_dram_parameter("ident", [D, D], BF16, isOutput=False)
    out = nc.declare_dram_parameter("out", [D, HW], F32, isOutput=True)

    HH = H // 2  # corr DMA half-chunk rows

    with tile.TileContext(nc) as tc:
        with (
            tc.tile_pool(name="const", bufs=1) as cpool,
            tc.tile_pool(name="stage", bufs=2) as spool,
            tc.tile_pool(name="work", bufs=3) as wpool,
            tc.tile_pool(name="qpool", bufs=4) as qpool,
            tc.tile_pool(name="xpool", bufs=7) as xpool,
            tc.tile_pool(name="epool", bufs=9) as epool,
            tc.tile_pool(name="ps_s", bufs=2, space="PSUM") as ps_s,
            tc.tile_pool(name="ps_av", bufs=2, space="PSUM") as ps_av,
            tc.tile_pool(name="ps_c", bufs=1, space="PSUM") as ps_cp,
            tc.tile_pool(name="ps_m", bufs=1, space="PSUM") as ps_mp,
        ):
            # ---- memsets first so nothing queues behind DMA descriptors ----
            warm = cpool.tile([128, 128], BF16)
            nc.vector.memset(warm[:], 0.0)
            corr_pad = []
            for c in range(3):
                cp = cpool.tile([KC, H + 2, W + 2], BF16, name=f"corr_pad{c}")
                nc.vector.memset(cp[:, 0, :], 0.0)
                nc.vector.memset(cp[:, H + 1, :], 0.0)
                nc.vector.memset(cp[:, 1:H + 1, 0:1], 0.0)
                nc.vector.memset(cp[:, 1:H + 1, W + 1:W + 2], 0.0)
                corr_pad.append(cp)
            ones_f8 = cpool.tile([128, 1], F8)
            nc.gpsimd.memset(ones_f8[:], 1.0)
            ones_row = cpool.tile([1, TI], BF16)
            nc.gpsimd.memset(ones_row[:], 1.0)
            ebias_sb = cpool.tile([128, 1], F32)
            nc.gpsimd.memset(ebias_sb[:], EBIAS)
            # mask[r, ii, :] = 1 iff row r holds an ii-denominator partial
            # (ii0 partials live in rows {0,32}, ii1 in rows {64,96})
            mask = cpool.tile([128, 2, 128], BF16)
            nc.gpsimd.memset(mask[:], 0.0)
            for t in range(2):
                for ii in range(2):
                    r = 64 * ii + 32 * t
                    nc.vector.memset(mask[r:r + 1, ii, :], 1.0)
            # preload the exp table set during the DMA wait
            tdummy = cpool.tile([1, 16], BF16)
            nc.scalar.activation(tdummy[:], warm[0:1, 0:16], AF.Exp)

            # ---- input DMAs: wskT chunks interleave with corr halves so
            # the first conv matmuls start as early as possible ----
            wskT_sb = cpool.tile([KC, 27, D], BF16)
            stgs = []
            for c in range(3):
                nc.sync.dma_start(wskT_sb[:, c * 9:(c + 1) * 9, :],
                                  wskT[:, c * 9:(c + 1) * 9, :])
                stg = spool.tile([KC, HH * W], BF16, name="stg", bufs=3)
                nc.sync.dma_start(stg[:], corr[c * KC:(c + 1) * KC, 0:HH * W])
                nc.vector.tensor_copy(
                    corr_pad[c][:, 1:HH + 1, 1:W + 1],
                    stg.rearrange("p (h w) -> p h w", h=HH))
            k_sb = cpool.tile([D, HW], BF16)
            nc.sync.dma_start(k_sb[:], k_in[:])
            for c in range(3):
                stg = spool.tile([KC, HH * W], BF16, name="stg", bufs=3)
                nc.sync.dma_start(stg[:],
                                  corr[c * KC:(c + 1) * KC, HH * W:HW])
                nc.vector.tensor_copy(
                    corr_pad[c][:, HH + 1:H + 1, 1:W + 1],
                    stg.rearrange("p (h w) -> p h w", h=HH))
            # vT2_sb[p, jp, jj, d] = v[d, (2*jp+jj)*128 + p] in fp8
            vT2_sb = cpool.tile([128, NJP, 2, D], F8)
            nc.sync.dma_start(vT2_sb[:], vT2[:])
            wprojT_sb = cpool.tile([D, 2, D], BF16)
            nc.gpsimd.dma_start(wprojT_sb[:], wprojT.rearrange("c p d -> p c d"))
            wf1T_sb = cpool.tile([D, D], BF16)
            nc.gpsimd.dma_start(wf1T_sb[:], wf1T[:])
            wf2T_sb = cpool.tile([D, D], BF16)
            nc.gpsimd.dma_start(wf2T_sb[:], wf2T[:])
            ident_sb = cpool.tile([D, D], BF16)
            nc.gpsimd.dma_start(ident_sb[:], ident[:])
            b_sk_sb = cpool.tile([1, D], BF16)
            nc.gpsimd.dma_start(b_sk_sb[:], b_sk[:])
            b_proj_sb = cpool.tile([1, D], BF16)
            nc.gpsimd.dma_start(b_proj_sb[:], b_proj[:])
            b_f1_sb = cpool.tile([D, 1], F32)
            nc.gpsimd.dma_start(b_f1_sb[:], b_f1[:])
            b_f2_sb = cpool.tile([1, D], BF16)
            nc.gpsimd.dma_start(b_f2_sb[:], b_f2[:])

            # HAM warm-up: keep PE busy while the first corr half lands
            ps_w = ps_cp.tile([128, 128], F32, name="ps_w", tag="c")
            for _ in range(40):
                nc.tensor.matmul(ps_w[:], warm[:], warm[:],
                                 start=True, stop=True)

            def conv_gen(p):
                """3x3 conv for i-tiles (2p, 2p+1), one [D,TI] psum bank,
                ii-serial. Also computes resid2 = conv + b_sk + b_proj
                + Wp1 @ q (the proj residual half, so the boundary only
                adds the normalized attention term). Yields None after
                each PE matmul, then yields [(resid2, q), ...] forever."""
                outs = []
                for ii in range(2):
                    ps_c = ps_cp.tile([D, TI], F32, name="ps_c", tag="c")
                    y0 = (2 * p + ii) * RT
                    for c in range(3):
                        for t in range(9):
                            dy, dx = t // 3, t % 3
                            nc.tensor.matmul(
                                ps_c[:],
                                wskT_sb[:, c * 9 + t, :],
                                corr_pad[c][:, y0 + dy:y0 + dy + RT,
                                            dx:dx + W],
                                start=(c == 0 and t == 0),
                                stop=(c == 2 and t == 8 and not with_bias))
                            yield None
                    if with_bias:
                        nc.tensor.matmul(ps_c[:], b_sk_sb[:], ones_row[:],
                                         start=False, stop=True)
                        yield None
                    q = qpool.tile([D, TI], BF16, name="q", tag="q")
                    nc.vector.tensor_copy(q[:], ps_c[:])
                    resid = qpool.tile([D, TI], F32, name="resid", tag="r")
                    nc.vector.tensor_copy(resid[:], ps_c[:])
                    yield None
                    ps_r = ps_cp.tile([D, TI], F32, name="ps_r", tag="c")
                    nc.tensor.matmul(ps_r[:], wprojT_sb[:, 1, :], q[:],
                                     start=True, stop=not with_bias)
                    if with_bias:
                        nc.tensor.matmul(ps_r[:], b_proj_sb[:], ones_row[:],
                                         start=False, stop=True)
                    yield None
                    resid2 = qpool.tile([D, TI], F32, name="resid2", tag="r2")
                    nc.vector.tensor_add(resid2[:], ps_r[:], resid[:])
                    outs.append((resid2, q))
                    yield None
                while True:
                    yield outs

            def run_conv(gen):
                while True:
                    r = next(gen)
                    if r is not None:
                        return r

            def s_pair(j, qs):
                t = ps_s.tile([128, 2, TI], F32, name="ps_sj", tag="s")
                for ii in range(2):
                    nc.tensor.matmul(t[:, ii, :],
                                     k_sb[:, j * 128:(j + 1) * 128],
                                     qs[ii][:], start=True, stop=True)
                return t

            def ones_group(ps_m, g4, e2s):
                """denominator partial sums for e-tiles j=4*g4..4*g4+3:
                fp8 ones-matmuls col-packed 4-wide per j-pair tile;
                ii0 -> rows {0,32}, ii1 -> {64,96}. ps_m was
                zero-initialized by a start=True matmul, so always
                accumulate."""
                for t in range(2):
                    for ii in range(2):
                        for jj in range(2):
                            r = 64 * ii + 32 * jj
                            nc.tensor.matmul(
                                ps_m[r:r + 1, :], ones_f8[:, 0:1],
                                e2s[t][:, jj, ii, :],
                                start=False,
                                stop=(g4 == NJ // 4 - 1 and t == 1),
                                skip_group_check=True,
                                tile_position=(0, r))

            def new_ps_m():
                """fresh denominator bank, zeroed by a 0-weights matmul so
                the mask-merge never touches stale PSUM garbage."""
                ps_m = ps_mp.tile([128, TI], F32, name="ps_m", tag="m")
                nc.tensor.matmul(ps_m[:], warm[:], k_sb[:, 0:TI],
                                 start=True, stop=True)
                return ps_m

            def merge_recip(m4, ii):
                """rb = 1 / (mask-merged denominator broadcast)."""
                ps_b = ps_mp.tile([128, TI], F32, name="ps_b", tag="m")
                nc.tensor.matmul(ps_b[:], mask[:, ii, :], m4[:],
                                 start=True, stop=True)
                rb = wpool.tile([128, TI], F32, name="rb", tag="rb")
                nc.vector.reciprocal(rb[:], ps_b[:])
                return rb

            def proj_av(av_ii):
                ps_p = ps_mp.tile([D, TI], F32, name="ps_p", tag="m")
                nc.tensor.matmul(ps_p[:], wprojT_sb[:, 0, :], av_ii[:],
                                 start=True, stop=True)
                return ps_p

            def finish_x(ps_p, rb, resid2):
                t = wpool.tile([D, TI], F32, name="t", tag="t")
                nc.vector.tensor_mul(t[:], ps_p[:], rb[:])
                x_bf = xpool.tile([D, TI], BF16, name="x_bf", tag="xb")
                nc.vector.tensor_add(x_bf[:], t[:], resid2[:])
                return x_bf

            def ffn_head(x_bf):
                ps_f1 = ps_s.tile([D, TI], F32, name="ps_f1", tag="s")
                nc.tensor.matmul(ps_f1[:], wf1T_sb[:], x_bf[:],
                                 start=True, stop=True)
                h1 = wpool.tile([D, TI], BF16, name="h1", tag="h1")
                nc.scalar.activation(h1[:], ps_f1[:], AF.Gelu, bias=b_f1_sb[:])
                ps_f2 = ps_s.tile([D, TI], F32, name="ps_f2", tag="s")
                nc.tensor.matmul(ps_f2[:], ident_sb[:], x_bf[:],
                                 start=True, stop=False)
                nc.tensor.matmul(ps_f2[:], wf2T_sb[:], h1[:],
                                 start=False, stop=not with_bias)
                if with_bias:
                    nc.tensor.matmul(ps_f2[:], b_f2_sb[:], ones_row[:],
                                     start=False, stop=True)
                return ps_f2

            def ffn_tail(ps_f2, i):
                o = wpool.tile([D, TI], F32, name="o", tag="o")
                nc.vector.tensor_copy(o[:], ps_f2[:])
                nc.sync.dma_start(out[:, i * TI:(i + 1) * TI], o[:])

            # ---- prologue: conv pair 0 + S(0), S(1) ----
            rq = run_conv(conv_gen(0))
            qpair = [rq[0][1], rq[1][1]]
            ps_s0 = s_pair(0, qpair)
            ps_s1 = s_pair(1, qpair)

            xs = [None] * NT
            prev = None              # (m4, av, rq) of pair p-1
            for p in range(NP):
                last = p == NP - 1
                cgen = conv_gen(p + 1) if not last else None
                cdone = None
                qnext = None
                # boundary leftovers from pair p-1, injected into early j's
                inject = []
                if prev is not None:
                    pm4, pav, prq = prev
                    rbs = [None, None]

                    def mk_merge(ii):
                        def go():
                            rbs[ii] = merge_recip(pm4, ii)
                        return go

                    def mk_x(ii, pp=p):
                        def go():
                            ps_p = proj_av(pav[ii])
                            xs[2 * (pp - 1) + ii] = finish_x(
                                ps_p, rbs[ii], prq[ii][0])
                        return go
                    inject = [mk_merge(0), mk_merge(1), mk_x(0), mk_x(1)]

                ps_m = None
                ps_sj, ps_snx = ps_s0, ps_s1
                e2 = None
                e2_hist = []
                ps_a = [ps_av.tile([D, TI], F32, name=f"ps_a{ii}", tag="av")
                        for ii in range(2)]
                for j in range(NJ):
                    jp, jj = j // 2, j % 2
                    if jj == 0:
                        e2 = epool.tile([128, 2, 2, TI], F8, name="e2",
                                        tag="e")
                        e2_hist.append(e2)
                    # Act: the pacing instruction
                    nc.scalar.activation(e2[:, jj, :, :], ps_sj[:], AF.Exp,
                                         scale=SCALE, bias=ebias_sb[:])
                    # PE work for this j, in rough priority order
                    if j < NJ - 2:
                        ps_sj, ps_snx = ps_snx, s_pair(j + 2, qpair)
                    elif j == NJ - 2:
                        ps_sj = ps_snx
                        if not last:
                            cdone = run_conv(cgen)
                            cgen = None
                            qnext = [cdone[0][1], cdone[1][1]]
                            ps_s0 = s_pair(0, qnext)
                    else:
                        if not last:
                            ps_s1 = s_pair(1, qnext)
                    if jj == 1:
                        for ii in range(2):
                            nc.tensor.matmul(
                                ps_a[ii][:], vT2_sb[:, jp, :, :],
                                e2[:, :, ii, :],
                                start=(jp == 0), stop=(jp == NJP - 1),
                                perf_mode=DR)
                    # denominator groups: deferred so the 'm' bank is free
                    # of the previous boundary's merge/proj chain
                    if j in (12, 14, 16, 18, 20):
                        g4 = (j - 12) // 2
                        if ps_m is None:
                            ps_m = new_ps_m()
                        ones_group(ps_m, g4,
                                   e2_hist[2 * g4:2 * g4 + 2])
                    if inject:
                        inject.pop(0)()
                    if cgen is not None:
                        for _ in range(3):
                            next(cgen)
                # ---- boundary: finish denominators, evacuate AV ----
                ones_group(ps_m, NJ // 4 - 1, e2_hist[-2:])
                m4 = wpool.tile([128, TI], BF16, name="m4", tag="m4")
                nc.vector.tensor_copy(m4[:], ps_m[:])
                av = []
                for ii in range(2):
                    a = qpool.tile([D, TI], BF16, name="av_sb", tag="avs")
                    nc.vector.tensor_copy(a[:], ps_a[ii][:])
                    av.append(a)
                prev = (m4, av, rq)
                if not last:
                    rq = cdone
                    qpair = qnext

            # ---- tail: pair-2 normalize + all 6 FFNs (gelus after all
            # exps: one table switch; recips prioritized on the DVE) ----
            m4, av, rq2 = prev
            f2a = ffn_head(xs[0])
            f2b = ffn_head(xs[1])
            rb0 = merge_recip(m4, 0)
            rb1 = merge_recip(m4, 1)
            xs[4] = finish_x(proj_av(av[0]), rb0, rq2[0][0])
            ffn_tail(f2a, 0)
            ffn_tail(f2b, 1)
            f2c = ffn_head(xs[2])
            f2d = ffn_head(xs[3])
            xs[5] = finish_x(proj_av(av[1]), rb1, rq2[1][0])
            f2e = ffn_head(xs[4])
            ffn_tail(f2c, 2)
            ffn_tail(f2d, 3)
            f2f = ffn_head(xs[5])
            ffn_tail(f2e, 4)
            ffn_tail(f2f, 5)

    _split_multi_waits(nc)
    return nc


_NC = {}


def _get_nc(with_bias=True):
    if with_bias not in _NC:
        _NC[with_bias] = build_nc(with_bias)
    return _NC[with_bias]


def _prep_core(corr, k, v, w_sk, b_sk, w_proj, b_proj, w_ffn1, b_ffn1,
               w_ffn2, b_ffn2):
    bf = ml_dtypes.bfloat16
    f8 = ml_dtypes.float8_e4m3
    wskT = np.empty((KC, 27, D), dtype=bf)
    for c in range(3):
        for t in range(9):
            dy, dx = t // 3, t % 3
            wskT[:, c * 9 + t, :] = \
                w_sk[:, c * KC:(c + 1) * KC, dy, dx].T.astype(bf)
    vT = v.reshape(D, HW).T.reshape(NJ, 128, D).transpose(1, 0, 2)
    vT2 = np.ascontiguousarray(vT).astype(f8).reshape(128, NJP, 2, D)
    return {
        "corr": corr.reshape(CIN, HW).astype(bf),
        "k": k.reshape(D, HW).astype(bf),
        "vT2": vT2,
        "wskT": wskT,
        "b_sk": b_sk.reshape(1, D).astype(bf),
        "wprojT": np.ascontiguousarray(
            w_proj.reshape(D, 2 * D).T.reshape(2, D, D)).astype(bf),
        "b_proj": b_proj.reshape(1, D).astype(bf),
        "wf1T": np.ascontiguousarray(w_ffn1.reshape(D, D).T).astype(bf),
        "b_f1": b_ffn1.reshape(D, 1).astype(np.float32),
        "wf2T": np.ascontiguousarray(w_ffn2.reshape(D, D).T).astype(bf),
        "b_f2": b_ffn2.reshape(1, D).astype(bf),
        "ident": np.eye(D, dtype=bf),
    }


def make_in_maps(corr, k, v, w_sk, b_sk, w_proj, b_proj, w_ffn1, b_ffn1,
                 w_ffn2, b_ffn2):
    corr = np.asarray(corr, dtype=np.float32)
    k = np.asarray(k, dtype=np.float32)
    v = np.asarray(v, dtype=np.float32)
    return [
        _prep_core(corr[i], k[i], v[i], np.asarray(w_sk, np.float32),
                   np.asarray(b_sk, np.float32),
                   np.asarray(w_proj, np.float32),
                   np.asarray(b_proj, np.float32),
                   np.asarray(w_ffn1, np.float32),
                   np.asarray(b_ffn1, np.float32),
                   np.asarray(w_ffn2, np.float32),
                   np.asarray(b_ffn2, np.float32))
        for i in range(N)
    ]


def kernel(corr, k, v, w_sk, b_sk, w_proj, b_proj, w_ffn1, b_ffn1,
           w_ffn2, b_ffn2):
    with_bias = bool(np.any(np.asarray(b_proj)) or np.any(np.asarray(b_ffn2))
                     or np.any(np.asarray(b_sk)))
    nc = _get_nc(with_bias)
    in_maps = make_in_maps(corr, k, v, w_sk, b_sk, w_proj, b_proj,
                           w_ffn1, b_ffn1, w_ffn2, b_ffn2)
    res = run_bass_kernel_spmd(nc, in_maps, list(range(N)))
    out = np.stack([res.results[i]["out"].reshape(D, H, W) for i in range(N)])
    return out.astype(np.float32)
